# revision 15
# baseline (speedup 1.0000x reference)
"""Trainium2 Bass kernel for nn_Decoder (relational GCN decoder with cross-attention).

Strategy (8 NeuronCores, SPMD):
  - Shard target nodes N=50000 -> 6250/core (padded 6272 = 49*128 windows).
  - Each core keeps its h shard TRANSPOSED in SBUF (hT: [256 emb (2 part-tiles), nodes]).
  - Per layer:
      A) per-type tables h_t = h @ W_rel[t] (node-major rows, 4 types interleaved)
         -> local DRAM -> AllGather -> full 200704-row table.
      B) cross-attention: scoresT = kT-chunks.T @ hT, exp (no max; scores are tiny),
         Z via ones-matmul over partitions, ctxT = k-chunks.T @ expT, 1/Z broadcast
         via K=1 matmul; ctx written into the next-h buffer.
      C) messages: dma_gather per source-rank slice of the full table (int16 ids)
         -> DRAM staging -> dma_gather in target-window chunk order ->
         segment-sum via one-hot matmuls (iota-compare) accumulated in PSUM
         together with W_h @ h; h_new = relu(agg + ctx).
  - Heads: node log-softmax from hT3; edge head via per-node z = h3 @ [Wg|Wg] + bg
    rows (64 f32, 10 used), AllGathered, dma_gathered per edge endpoint.
All matmuls bf16 with f32 PSUM accumulation.
"""

import numpy as np
import ml_dtypes

import concourse.bass as bass
import concourse.mybir as mybir
import concourse.tile as tile
from concourse import bacc, bass_utils

BF16 = mybir.dt.bfloat16
F32 = mybir.dt.float32
I16 = mybir.dt.int16
I32 = mybir.dt.int32

nbf = ml_dtypes.bfloat16

# ---- problem dims (hardcoded) ----
P = 128
NCORES = 8
N, M, L, E = 50000, 1024, 3, 200000
HID, EMB, VOCAB = 512, 256, 512
NSLT, NSRT = 4, 5
NSH = N // NCORES            # 6250
NWIN = 49
NSHP = NWIN * P              # 6272
NBLK = 13                    # 12 blocks of 512 + 1 of 128
BLKW = [512] * 12 + [128]
TBL_LOC = NSHP * NSLT        # 25088 rows per rank in the h_t table
TBL_FULL = TBL_LOC * NCORES  # 200704
ZW = 64                      # z row width (f32, 256B; 10 cols used)
INV_SQRT_EMB = 1.0 / 16.0

_cached = {}


def _wrap16(lst, S):
    """Pack an int list into the dma_gather [128, S] int16 index tile."""
    out = np.zeros((P, S), np.int16)
    i = np.arange(len(lst))
    out[i % 16, i // 16] = np.asarray(lst, np.int16)
    return out


# ------------------------------------------------------------------
# host-side routing / input prep
# ------------------------------------------------------------------
def _prep_host(tgt_x, x, tgt_edge_index, tgt_edge_type, embed_table,
               W_h1, W_rel1, W_src1, W_h3, W_rel3, W_src3, Wz, bz, Wg, bg):
    tgt_x = np.asarray(tgt_x)
    src = np.asarray(tgt_edge_index[0]).astype(np.int64)
    tgt = np.asarray(tgt_edge_index[1]).astype(np.int64)
    ety = np.asarray(tgt_edge_type).astype(np.int64)

    groups = {}
    counts_win = np.zeros((NCORES, NWIN), np.int64)
    for c in range(NCORES):
        sel = np.nonzero(tgt // NSH == c)[0]
        tloc = tgt[sel] - c * NSH
        w = tloc // P
        order = np.lexsort((sel, w))
        groups[c] = dict(sel=sel[order], src=src[sel][order], tloc=tloc[order],
                         ety=ety[sel][order], w=w[order])
        counts_win[c] = np.bincount(w, minlength=NWIN)

    ncw = np.maximum(1, -(-counts_win.max(axis=0) // P))
    ncw_off = np.concatenate([[0], np.cumsum(ncw)]).astype(np.int64)
    NCT = int(ncw_off[-1])
    E_cap = int(counts_win.sum(axis=1).max())
    CZ = -(-E_cap // P)

    wcat = np.zeros((EMB, 16), np.float32)
    wcat[:, 0:5] = Wg[:EMB]
    wcat[:, 5:10] = Wg[EMB:]
    bgrow = np.zeros((1, 16), np.float32)
    bgrow[0, 5:10] = bg
    iota = np.tile(np.arange(P, dtype=np.int32)[None, :], (P, 1))

    common = {
        "embt": np.asarray(embed_table).astype(nbf),
        "xT": np.asarray(x).T.copy().astype(nbf),
        "wrel1": np.asarray(W_rel1).transpose(1, 0, 2).copy().astype(nbf),
        "wrel3": np.asarray(W_rel3).transpose(1, 0, 2).copy().astype(nbf),
        "wh1": np.asarray(W_h1).astype(nbf),
        "wh3": np.asarray(W_h3).astype(nbf),
        "wsrc1": np.asarray(W_src1).astype(nbf),
        "wsrc3": np.asarray(W_src3).astype(nbf),
        "wz": np.asarray(Wz).astype(nbf),
        "bzrow": np.asarray(bz).reshape(1, VOCAB).astype(nbf),
        "wgcat": wcat.astype(nbf),
        "bgrow": bgrow.astype(nbf),
        "cst_iota": iota,
        "cst_idt": np.eye(P, dtype=np.float32).astype(nbf),
        "cst_ones_col": np.ones((P, 1), nbf),
        "cst_ones_row": np.ones((1, P), nbf),
        "cst_ones_row_f32": np.ones((1, P), np.float32),
    }

    in_maps = []
    unpack = []
    for c in range(NCORES):
        g = groups[c]
        ecnt = len(g["sel"])
        # message gather: flat rows of ht_full, window-chunk slots
        flat = (g["src"] // NSH) * TBL_LOC + (g["src"] % NSH) * NSLT + g["ety"]
        wstart = np.concatenate([[0], np.cumsum(counts_win[c])])
        jw = np.arange(ecnt) - wstart[g["w"]]
        cols = (ncw_off[g["w"]] + jw // P).astype(np.int64)
        lanes = (jw % P).astype(np.int64)
        gidx = np.zeros((P, NCT), np.int32)
        trel = np.full((P, NCT), -1, np.int32)
        gidx[lanes, cols] = flat.astype(np.int32)
        trel[lanes, cols] = (g["tloc"] - g["w"] * P).astype(np.int32)
        # z gathers in dense (window-sorted) edge order; z_full row = rank*NSHP+local
        ezs = np.zeros((P, CZ), np.int32)
        ezt = np.zeros((P, CZ), np.int32)
        j = np.arange(ecnt)
        ezs[j % P, j // P] = ((g["src"] // NSH) * NSHP + g["src"] % NSH).astype(np.int32)
        ezt[j % P, j // P] = (c * NSHP + g["tloc"]).astype(np.int32)
        # embedding idx [128, NWIN*3]
        shard = np.zeros((NSHP, L), np.int64)
        shard[:NSH] = tgt_x[c * NSH:(c + 1) * NSH]
        embi = shard.reshape(NWIN, P, L).transpose(1, 0, 2).reshape(P, NWIN * L)

        mm = dict(common)
        mm.update({"embi": embi.astype(np.int32), "gidx": gidx, "trel": trel,
                   "ezs": ezs, "ezt": ezt})
        in_maps.append(mm)
        unpack.append(dict(sel=g["sel"], ecnt=ecnt))

    meta = dict(ncw=ncw, ncw_off=ncw_off, NCT=NCT, CZ=CZ, unpack=unpack)
    return in_maps, meta


# ------------------------------------------------------------------
# device program
# ------------------------------------------------------------------
def _build(tc, meta):
    nc = tc.nc
    ncw, ncw_off, NCT, CZ = meta["ncw"], meta["ncw_off"], meta["NCT"], meta["CZ"]

    din = {}
    for name, shape, dt in [
        ("embt", [VOCAB, EMB], BF16), ("xT", [HID, M], BF16),
        ("wrel1", [EMB, NSLT, EMB], BF16), ("wrel3", [EMB, NSLT, EMB], BF16),
        ("wh1", [EMB, EMB], BF16), ("wh3", [EMB, EMB], BF16),
        ("wsrc1", [HID, EMB], BF16), ("wsrc3", [HID, EMB], BF16),
        ("wz", [EMB, VOCAB], BF16), ("bzrow", [1, VOCAB], BF16),
        ("wgcat", [EMB, 16], BF16), ("bgrow", [1, 16], BF16),
        ("cst_iota", [P, P], I32), ("cst_idt", [P, P], BF16),
        ("cst_ones_col", [P, 1], BF16), ("cst_ones_row", [1, P], BF16),
        ("cst_ones_row_f32", [1, P], F32),
        ("embi", [P, NWIN * L], I32),
        ("gidx", [P, NCT], I32), ("trel", [P, NCT], I32),
        ("ezs", [P, CZ], I32), ("ezt", [P, CZ], I32),
    ]:
        din[name] = nc.dram_tensor(name, shape, dt, kind="ExternalInput").ap()
    node_out = nc.dram_tensor("node_out", [NSHP, VOCAB], F32, kind="ExternalOutput").ap()
    edge_out = nc.dram_tensor("edge_out", [P, CZ, NSRT], F32, kind="ExternalOutput").ap()

    dram = tc.alloc_tile_pool(name="dram", bufs=1, space="DRAM")
    ht_local = dram.tile([NWIN, P, NSLT, EMB], BF16, name="ht_local")
    ht_fulls = [dram.tile([TBL_FULL, EMB], BF16, addr_space="Shared",
                          tag=f"htf{i}", name=f"htf{i}") for i in range(3)]
    z_local = dram.tile([NSHP, 16], BF16, name="z_local")
    z_full = dram.tile([NSHP * NCORES, 16], BF16, addr_space="Shared", name="z_full")

    wp = tc.alloc_tile_pool(name="wp", bufs=1)
    hp = tc.alloc_tile_pool(name="hp", bufs=1)
    sb = tc.alloc_tile_pool(name="sb", bufs=2)
    ps = tc.alloc_tile_pool(name="ps", bufs=1, space="PSUM")

    def wtile(name, shape, dt, src_ap):
        t = wp.tile(shape, dt, tag=name, name=name)
        nc.sync.dma_start(out=t[:], in_=src_ap)
        return t

    xT = [wtile(f"xT{h}", [P, M], BF16, din["xT"][h * P:(h + 1) * P]) for h in range(4)]
    wrel, wh, wsrc = {}, {}, {}
    for ln in ("1", "3"):
        wrel[ln] = [wtile(f"wrel{ln}_{d}", [P, NSLT, EMB], BF16,
                          din[f"wrel{ln}"][d * P:(d + 1) * P]) for d in range(2)]
        wh[ln] = [wtile(f"wh{ln}_{d}", [P, EMB], BF16,
                        din[f"wh{ln}"][d * P:(d + 1) * P]) for d in range(2)]
        wsrc[ln] = [wtile(f"wsrc{ln}_{h}", [P, EMB], BF16,
                          din[f"wsrc{ln}"][h * P:(h + 1) * P]) for h in range(4)]
    wz = [wtile(f"wz{d}", [P, VOCAB], BF16, din["wz"][d * P:(d + 1) * P]) for d in range(2)]
    bzrow = wtile("bzrow", [1, VOCAB], BF16, din["bzrow"][:])
    wg = [wtile(f"wg{d}", [P, 16], BF16, din["wgcat"][d * P:(d + 1) * P]) for d in range(2)]
    bgrow = wtile("bgrow", [1, 16], BF16, din["bgrow"][:])
    iota = wtile("iota", [P, P], I32, din["cst_iota"][:])
    ones_col = wtile("ones_col", [P, 1], BF16, din["cst_ones_col"][:])
    ones_row = wtile("ones_row", [1, P], BF16, din["cst_ones_row"][:])
    ones_row_f32 = wtile("ones_row_f32", [1, P], F32, din["cst_ones_row_f32"][:])
    embi = wtile("embi", [P, NWIN * L], I32, din["embi"][:])
    gidx = wtile("gidx", [P, NCT], I32, din["gidx"][:])
    trel = wtile("trel", [P, NCT], I32, din["trel"][:])
    ezs = wtile("ezs", [P, CZ], I32, din["ezs"][:])
    ezt = wtile("ezt", [P, CZ], I32, din["ezt"][:])

    hT = [[[hp.tile([P, BLKW[b]], BF16, tag=f"h{s}_{d}_{b}", name=f"h{s}_{d}_{b}")
            for b in range(NBLK)] for d in range(2)] for s in range(2)]
    kt = [wp.tile([P, M], BF16, tag=f"kt{d}", name=f"kt{d}") for d in range(2)]
    kn = wp.tile([P, 8, EMB], BF16, tag="kn", name="kn")

    def win_ap(s, d, w):
        b, sub = divmod(w, 4)
        return hT[s][d][b][:, sub * P:(sub + 1) * P]

    def gather_rows(idx_col_ap, table_ap, width, tag, bufs=4):
        g = sb.tile([P, width], BF16, tag=tag, name=tag, bufs=bufs)
        nc.gpsimd.indirect_dma_start(
            out=g[:], out_offset=None, in_=table_ap,
            in_offset=bass.IndirectOffsetOnAxis(ap=idx_col_ap, axis=0))
        return g

    # ---------- embedding (transposed via PE) ----------
    idtb = wtile("idtb", [P, P], BF16, din["cst_idt"][:])
    for w in range(NWIN):
        g0 = gather_rows(embi[:, w * L:w * L + 1], din["embt"], EMB, "embg")
        g1 = gather_rows(embi[:, w * L + 1:w * L + 2], din["embt"], EMB, "embg")
        g2 = gather_rows(embi[:, w * L + 2:w * L + 3], din["embt"], EMB, "embg")
        h0 = sb.tile([P, EMB], BF16, tag="emb0", name="emb0", bufs=3)
        nc.vector.tensor_add(out=h0[:], in0=g0[:], in1=g1[:])
        nc.vector.tensor_add(out=h0[:], in0=h0[:], in1=g2[:])
        for d in range(2):
            pt = ps.tile([P, P], BF16, tag="psB", name="psB", bufs=3)
            nc.tensor.transpose(pt[:], h0[:, d * P:(d + 1) * P], idtb[:])
            nc.vector.tensor_copy(out=win_ap(0, d, w), in_=pt[:])

    # ---------- layers ----------
    def build_layer(cur, new, ln, ht_full):
        # k prep
        for d in range(2):
            for half in range(2):
                pk = ps.tile([P, 512], F32, tag="psA", name="psA", bufs=2)
                for h in range(4):
                    nc.tensor.matmul(pk[:], lhsT=wsrc[ln][h][:, d * P:(d + 1) * P],
                                     rhs=xT[h][:, half * 512:(half + 1) * 512],
                                     start=(h == 0), stop=(h == 3))
                nc.vector.tensor_copy(out=kt[d][:, half * 512:(half + 1) * 512], in_=pk[:])
        for mt in range(8):
            pk = ps.tile([P, EMB], F32, tag="psA", name="psA", bufs=2)
            for h in range(4):
                nc.tensor.matmul(pk[:], lhsT=xT[h][:, mt * P:(mt + 1) * P],
                                 rhs=wsrc[ln][h][:], start=(h == 0), stop=(h == 3))
            nc.vector.tensor_copy(out=kn[:, mt], in_=pk[:])

        # phase A: h_t tables -> DRAM -> AllGather
        for w in range(NWIN):
            pa = ps.tile([P, NSLT, EMB], F32, tag="psA", name="psA", bufs=2)
            for t in range(NSLT):
                for d in range(2):
                    nc.tensor.matmul(pa[:, t], lhsT=win_ap(cur, d, w),
                                     rhs=wrel[ln][d][:, t],
                                     start=(d == 0), stop=(d == 1))
            hsb = sb.tile([P, NSLT, EMB], BF16, tag="htsb", name="htsb", bufs=3)
            nc.vector.tensor_copy(out=hsb[:], in_=pa[:])
            nc.sync.dma_start(out=ht_local[w], in_=hsb[:])
        nc.gpsimd.collective_compute(
            "AllGather", mybir.AluOpType.bypass,
            replica_groups=[list(range(NCORES))],
            ins=[ht_local[:].opt()], outs=[ht_full[:].opt()])

        # phase B: attention
        for b in range(NBLK):
            nsubs = max(1, BLKW[b] // 256)
            subw = min(256, BLKW[b])
            for s_i in range(nsubs):
                ssl = slice(s_i * 256, s_i * 256 + subw)
                expT = sb.tile([P, 8, 256], BF16, tag="expT", name="expT", bufs=2)
                for mg in range(4):
                    pss = ps.tile([P, 2, 256], F32, tag="psB", name="psB", bufs=3)
                    for i in range(2):
                        mt = mg * 2 + i
                        for d in range(2):
                            nc.tensor.matmul(pss[:, i, :subw],
                                             lhsT=kt[d][:, mt * P:(mt + 1) * P],
                                             rhs=hT[cur][d][b][:, ssl],
                                             start=(d == 0), stop=(d == 1))
                        nc.scalar.activation(out=expT[:, mt, :subw], in_=pss[:, i, :subw],
                                             func=mybir.ActivationFunctionType.Exp,
                                             scale=INV_SQRT_EMB)
                pz = ps.tile([1, 256], F32, tag="psZ", name="psZ", bufs=1)
                for mt in range(8):
                    nc.tensor.matmul(pz[:, :subw], lhsT=ones_col[:],
                                     rhs=expT[:, mt, :subw],
                                     start=(mt == 0), stop=(mt == 7))
                rz = sb.tile([1, 256], F32, tag="rz", name="rz", bufs=2)
                nc.vector.reciprocal(out=rz[:, :subw], in_=pz[:, :subw])
                pzb = ps.tile([P, 256], F32, tag="psB", name="psB", bufs=3)
                nc.tensor.matmul(pzb[:, :subw], lhsT=ones_row_f32[:], rhs=rz[:, :subw],
                                 start=True, stop=True)
                zb = sb.tile([P, 256], F32, tag="zb", name="zb", bufs=2)
                nc.vector.tensor_copy(out=zb[:, :subw], in_=pzb[:, :subw])
                pc = ps.tile([P, 2, 256], F32, tag="psB", name="psB", bufs=3)
                for f in range(2):
                    for mt in range(8):
                        nc.tensor.matmul(pc[:, f, :subw],
                                         lhsT=kn[:, mt, f * P:(f + 1) * P],
                                         rhs=expT[:, mt, :subw],
                                         start=(mt == 0), stop=(mt == 7))
                for f in range(2):
                    nc.vector.tensor_mul(out=hT[new][f][b][:, ssl],
                                         in0=pc[:, f, :subw], in1=zb[:, :subw])

        # phase C: per-chunk gather + scatter
        for b in range(NBLK):
            w0, w1 = b * 4, min(b * 4 + 4, NWIN)
            for w in range(w0, w1):
                sub = w - b * 4
                wsl = slice(sub * P, sub * P + P)
                pagg = ps.tile([P, 2, P], F32, tag="psB", name="psB", bufs=3)
                for f in range(2):
                    for d in range(2):
                        nc.tensor.matmul(pagg[:, f],
                                         lhsT=wh[ln][d][:, f * P:(f + 1) * P],
                                         rhs=hT[cur][d][b][:, wsl],
                                         start=(d == 0), stop=False)
                nchunk = int(ncw[w])
                for cch in range(nchunk):
                    cc = int(ncw_off[w]) + cch
                    gt2 = gather_rows(gidx[:, cc:cc + 1], ht_full[:], EMB, "g2o", bufs=6)
                    oh = sb.tile([P, P], BF16, tag="oh", name="oh", bufs=4)
                    nc.vector.tensor_tensor(
                        out=oh[:],
                        in0=trel[:, cc:cc + 1].to_broadcast([P, P]),
                        in1=iota[:], op=mybir.AluOpType.is_equal)
                    for f in range(2):
                        nc.tensor.matmul(pagg[:, f], lhsT=gt2[:, f * P:(f + 1) * P],
                                         rhs=oh[:], start=False,
                                         stop=(cch == nchunk - 1))
                for f in range(2):
                    nc.vector.tensor_add(out=hT[new][f][b][:, wsl],
                                         in0=pagg[:, f], in1=hT[new][f][b][:, wsl])
            for f in range(2):
                nc.scalar.activation(out=hT[new][f][b][:], in_=hT[new][f][b][:],
                                     func=mybir.ActivationFunctionType.Relu)

    build_layer(0, 1, "1", ht_fulls[0])
    build_layer(1, 0, "1", ht_fulls[1])
    build_layer(0, 1, "3", ht_fulls[2])
    h3 = 1

    # ---------- node head + z head ----------
    for w in range(NWIN):
        b, sub = divmod(w, 4)
        wsl = slice(sub * P, sub * P + P)
        pn = ps.tile([P, VOCAB], F32, tag="psB", name="psB", bufs=3)
        for d in range(2):
            nc.tensor.matmul(pn[:], lhsT=hT[h3][d][b][:, wsl], rhs=wz[d][:],
                             start=(d == 0), stop=False)
        nc.tensor.matmul(pn[:], lhsT=ones_row[:], rhs=bzrow[:], start=False, stop=True)
        negmax = sb.tile([P, 1], F32, tag="negmax", name="negmax", bufs=2)
        nc.vector.tensor_reduce(out=negmax[:], in_=pn[:], op=mybir.AluOpType.max,
                                axis=mybir.AxisListType.X, negate=True)
        ex = sb.tile([P, VOCAB], F32, tag="nh_ex", name="nh_ex", bufs=2)
        nc.scalar.activation(out=ex[:], in_=pn[:], func=mybir.ActivationFunctionType.Exp,
                             bias=negmax[:, 0:1], scale=1.0)
        ssum = sb.tile([P, 1], F32, tag="nh_sum", name="nh_sum", bufs=2)
        nc.vector.tensor_reduce(out=ssum[:], in_=ex[:], op=mybir.AluOpType.add,
                                axis=mybir.AxisListType.X)
        lnz = sb.tile([P, 1], F32, tag="nh_lnz", name="nh_lnz", bufs=2)
        nc.scalar.activation(out=lnz[:], in_=ssum[:], func=mybir.ActivationFunctionType.Ln)
        shift = sb.tile([P, 1], F32, tag="nh_sh", name="nh_sh", bufs=2)
        nc.vector.tensor_sub(out=shift[:], in0=negmax[:], in1=lnz[:])
        outt = sb.tile([P, VOCAB], F32, tag="nh_out", name="nh_out", bufs=2)
        nc.vector.tensor_add(out=outt[:], in0=pn[:], in1=shift[:].to_broadcast([P, VOCAB]))
        nc.sync.dma_start(out=node_out[w * P:(w + 1) * P], in_=outt[:])

        pz2 = ps.tile([P, 16], F32, tag="psZ", name="psZ", bufs=1)
        for d in range(2):
            nc.tensor.matmul(pz2[:], lhsT=hT[h3][d][b][:, wsl], rhs=wg[d][:],
                             start=(d == 0), stop=False)
        nc.tensor.matmul(pz2[:], lhsT=ones_row[:], rhs=bgrow[:], start=False, stop=True)
        zsb = sb.tile([P, 16], BF16, tag="zsb", name="zsb", bufs=3)
        nc.vector.tensor_copy(out=zsb[:], in_=pz2[:])
        nc.sync.dma_start(out=z_local[w * P:(w + 1) * P], in_=zsb[:])

    nc.gpsimd.collective_compute(
        "AllGather", mybir.AluOpType.bypass,
        replica_groups=[list(range(NCORES))],
        ins=[z_local[:].opt()], outs=[z_full[:].opt()])

    # ---------- edge head (per-chunk z gathers) ----------
    for c0 in range(0, CZ, 8):
        cw = min(8, CZ - c0)
        el = sb.tile([P, 8, NSRT], F32, tag="el", name="el", bufs=2)
        for k in range(cw):
            gsr = gather_rows(ezs[:, c0 + k:c0 + k + 1], z_full[:], 16, "zg_s")
            gtr = gather_rows(ezt[:, c0 + k:c0 + k + 1], z_full[:], 16, "zg_t")
            nc.vector.tensor_add(out=el[:, k], in0=gsr[:, 0:NSRT], in1=gtr[:, 5:5 + NSRT])
        enm = sb.tile([P, 8, 1], F32, tag="enm", name="enm", bufs=2)
        nc.vector.tensor_reduce(out=enm[:, :cw], in_=el[:, :cw], op=mybir.AluOpType.max,
                                axis=mybir.AxisListType.X, negate=True)
        nc.vector.tensor_add(out=el[:, :cw], in0=el[:, :cw],
                             in1=enm[:, :cw].to_broadcast([P, cw, NSRT]))
        eex = sb.tile([P, 8, NSRT], F32, tag="eex", name="eex", bufs=2)
        nc.scalar.activation(out=eex[:, :cw], in_=el[:, :cw],
                             func=mybir.ActivationFunctionType.Exp)
        esum = sb.tile([P, 8, 1], F32, tag="esum", name="esum", bufs=2)
        nc.vector.tensor_reduce(out=esum[:, :cw], in_=eex[:, :cw], op=mybir.AluOpType.add,
                                axis=mybir.AxisListType.X)
        elnz = sb.tile([P, 8, 1], F32, tag="elnz", name="elnz", bufs=2)
        nc.scalar.activation(out=elnz[:, :cw], in_=esum[:, :cw],
                             func=mybir.ActivationFunctionType.Ln)
        nc.vector.tensor_sub(out=el[:, :cw], in0=el[:, :cw],
                             in1=elnz[:, :cw].to_broadcast([P, cw, NSRT]))
        nc.sync.dma_start(out=edge_out[:, c0:c0 + cw], in_=el[:, :cw])

    for pool in (sb, ps, hp, wp, dram):
        pool.release()


# ------------------------------------------------------------------
# entry point
# ------------------------------------------------------------------
def _compile(meta):
    key = (meta["NCT"], meta["CZ"], tuple(int(v) for v in meta["ncw"]))
    if key in _cached:
        return _cached[key]
    nc = bacc.Bacc("TRN2", target_bir_lowering=False, debug=False, num_devices=NCORES)
    with tile.TileContext(nc) as tc:
        _build(tc, meta)
    nc.compile()
    _cached[key] = nc
    return nc


def kernel(run_opts=None, **inputs):
    in_maps, meta = _prep_host(**inputs)
    nc = _compile(meta)
    res = bass_utils.run_bass_kernel_spmd(
        nc, in_maps, core_ids=list(range(NCORES)), **(run_opts or {}))
    results = res.results

    node_pred = np.zeros((N, VOCAB), np.float32)
    edge_pred = np.zeros((E, NSRT), np.float32)
    for c in range(NCORES):
        node_pred[c * NSH:(c + 1) * NSH] = results[c]["node_out"][:NSH]
        u = meta["unpack"][c]
        eo = results[c]["edge_out"].transpose(1, 0, 2).reshape(-1, NSRT)[:u["ecnt"]]
        edge_pred[u["sel"]] = eo
    if run_opts:
        kernel.last_res = res
    return node_pred, edge_pred
